# revision 30
# baseline (speedup 1.0000x reference)
"""Trainium2 Bass kernel for nn_NonLinearReadoutLayer (equivariant gated MLP readout).

Reference computation (per node, N=200000):
    s = x[:, :128]; v = x[:, 128:].reshape(N, 128, 3)
    h_s = (s @ w1_s) / sqrt(128)                # [N, 256]
    h_v = einsum('nmc,mk->nkc', v, w1_v) / sqrt(128)
    act = silu(h_s[:, :128]); gates = sigmoid(h_s[:, 128:])
    out_s = (act @ w2_s) / sqrt(128)            # [N, 16]
    out_v = einsum('nmc,mk->nkc', h_v * gates[:,:,None], w2_v) / sqrt(128)
    out = concat([out_s, out_v.reshape(N, 48)], 1)   # [N, 64]

Strategy: pure data-parallel over nodes across 8 cores. Host-side marshalling
puts x in feature-major layout xt[f, n] with the vector part de-interleaved
(f = 128 + 128*c + m) and CAST TO BF16 (halves the input HBM traffic, which
was the binding roofline at fp32; end-to-end absmax rel err ~4e-3 vs the
2e-2 gate). 1/sqrt(128) is folded into the weights. All on-chip ops are
[128]-contraction matmuls with nodes on the moving/free axis.

Pipeline: per 512-node supertile k the chain is L1 matmuls -> tanh/silu ->
gate-mul -> L2 matmuls. The in-order PE stream is software-pipelined with a
one-supertile skew (L1(k) ... L2(k-1)) so PE never waits on the activation
chain. Gate sigmoid is tanh-form (same ScalarE LUT set as Silu -> no table
reloads; the 0.5 folds into the layer-2 vector weights). (tanh+1)*h_v is
fused scalar_tensor_tensor on DVE, as one [128,2,512] op over a contiguous
two-bank h_v pair (stride-0 broadcast of tanh) plus one single — fewer
fixed-cost PSUM accesses. The four L2 matmuls of two consecutive supertiles
accumulate into ONE [128,512] PSUM bank (partitions 0:64 / 64:128), halving
evacuation cost; each pair is evacuated bf16 (ScalarE) and DMA'd out.
PSUM budget: hv01 ring2 (4) + hv2 (1) + hsa (1) + hsg (1) + po (1) = 8 banks.
"""

import numpy as np
import ml_dtypes

import concourse.mybir as mybir
import concourse.tile as tile
from concourse import bacc
from concourse.bass_utils import run_bass_kernel_spmd

N_CORES = 8
P = 128
ST = 512  # nodes per matmul group (one PSUM bank of fp32)
MT = 2048  # nodes per DMA megatile
N_TOTAL = 200000
NC_NODES = N_TOTAL // N_CORES  # 25000
NP = 25088  # padded per-core nodes = 49 supertiles
N_ST = NP // ST  # 49
N_PAIR = (N_ST + 1) // 2  # 25 output pairs (last is half)

AF = mybir.ActivationFunctionType
ALU = mybir.AluOpType
BF16 = ml_dtypes.bfloat16

import os as _os

SPLIT_FIRST_MT = _os.environ.get("K_SPLIT_FIRST_MT", "1") == "1"
LDW_DEDUPE = _os.environ.get("K_LDW_DEDUPE", "0") == "1"


def _mm_noload(nc, out, lhsT, rhs, start, stop):
    """Matmul marked ldweights=False: PE array already holds lhsT from a
    preceding explicit nc.tensor.ldweights(lhsT). lhsT stays in ins so the
    dependency tracker still orders this after the weights are resident."""
    eng = nc.tensor
    ifmap_ap = eng.lower_ap(rhs.opt({0}), opt=False)
    weights_ap = eng.lower_ap(lhsT.opt({0}), opt=False, for_matmul_weights=True)
    out_ap = eng.lower_ap(out)
    return eng.add_instruction(
        mybir.InstMatmult(
            name=nc.get_next_instruction_name(),
            replication_resolution=0,
            replication_shift_amnt=0,
            replication_num_rows=0,
            start_tensor_calc=start,
            stop_tensor_calc=stop,
            ins=[ifmap_ap, weights_ap],
            outs=[out_ap],
            perf_mode=None,
            is_transpose=False,
            tile_position=(lhsT.base_partition(), out.base_partition()),
            tile_size=(128, 128),
            ldweights=False,
        )
    )

_CACHE = {}


def emit_body(nc, pools, xt_ap, out_ap, w):
    """One full pass over the node range. w is the preloaded weight tile."""
    f32 = mybir.dt.float32
    bf16 = mybir.dt.bfloat16
    inp, mid, osbp, ps = pools

    w1sa = w[:, 0:128]
    w1sb = w[:, 128:256]
    w1v = w[:, 256:384]
    w2 = [w[:, 384 + 64 * i : 448 + 64 * i] for i in range(4)]

    xt_r = xt_ap.rearrange("(b p) n -> p b n", p=P)

    state = {"po": None}
    skew = int(_os.environ.get("K_SKEW", "1"))

    def emit_l2(prev):
        act, gv01, gv2, k = prev
        if k % 2 == 0:
            state["po"] = ps.tile([P, ST], f32, tag="po", bufs=1, name="po")
        po = state["po"]
        prow = slice(64 * (k % 2), 64 * (k % 2) + 64)
        # accumulation order: act and gv2 first (their producers finish
        # earliest), the pair-fused gv01 last
        nc.tensor.matmul(po[prow, :], w2[0], act[:], start=True, stop=False)
        nc.tensor.matmul(po[prow, :], w2[3], gv2[:], start=False, stop=False)
        nc.tensor.matmul(po[prow, :], w2[1], gv01[:, 0, :], start=False, stop=False)
        nc.tensor.matmul(po[prow, :], w2[2], gv01[:, 1, :], start=False, stop=True)
        if k % 2 == 1 or k == N_ST - 1:
            pair = k // 2
            rows = 128 if k % 2 == 1 else 64
            opair = osbp.tile([P, ST], bf16, tag="opair", bufs=5, name="opair")
            nc.scalar.copy(opair[:rows, :], po[:rows, :])
            nc.scalar.dma_start(
                out=out_ap[0:rows, pair * ST : (pair + 1) * ST],
                in_=opair[:rows, :],
            )

    pending = []
    for m0 in range(0, NP, MT):
        mt = min(MT, NP - m0)
        xin = inp.tile([P, 4, mt], bf16, tag="xin")
        if m0 == 0 and SPLIT_FIRST_MT:
            # split only the first megatile's DMA so compute starts after
            # ~1.6us (first supertile chunk) instead of the full 6us transfer
            for c0 in range(0, mt, ST):
                nc.sync.dma_start(
                    out=xin[:, :, c0 : c0 + ST], in_=xt_r[:, :, c0 : c0 + ST]
                )
        else:
            nc.sync.dma_start(out=xin[:], in_=xt_r[:, :, m0 : m0 + mt])

        for s0 in range(0, mt, ST):
            k = (m0 + s0) // ST
            sl = slice(s0, s0 + ST)
            # --- layer 1 matmuls: gate half first so tanh starts early ---
            h_sg = ps.tile([P, ST], f32, tag="hsg", bufs=1, name="hsg")
            nc.tensor.matmul(h_sg[:], w1sb, xin[:, 0, sl], start=True, stop=True)
            h_sa = ps.tile([P, ST], f32, tag="hsa", bufs=1, name="hsa")
            nc.tensor.matmul(h_sa[:], w1sa, xin[:, 0, sl], start=True, stop=True)
            hv01 = ps.tile([P, 2, ST], f32, tag="hv01", bufs=2, name="hv01")
            nc.tensor.matmul(hv01[:, 0, :], w1v, xin[:, 1, sl], start=True, stop=True)
            nc.tensor.matmul(hv01[:, 1, :], w1v, xin[:, 2, sl], start=True, stop=True)
            hv2 = ps.tile([P, ST], f32, tag="hv2", bufs=1, name="hv2")
            nc.tensor.matmul(hv2[:], w1v, xin[:, 3, sl], start=True, stop=True)
            # --- activations: sigmoid(x) = 0.5*(tanh(x/2)+1); 0.5 in w2v ---
            th = mid.tile([P, ST], bf16, tag="th", bufs=5, name="th")
            nc.scalar.activation(th[:], h_sg[:], AF.Tanh, scale=0.5)
            act = mid.tile([P, ST], bf16, tag="act", bufs=5, name="act")
            nc.scalar.activation(act[:], h_sa[:], AF.Silu)
            # --- gv = (th + 1) * h_v, fused on DVE; single (hv2) first so
            # its ring-1 bank frees before the next iteration's matmul ---
            gv2 = mid.tile([P, ST], bf16, tag="gv2", bufs=5, name="gv2")
            nc.vector.scalar_tensor_tensor(
                gv2[:], th[:], 1.0, hv2[:], op0=ALU.add, op1=ALU.mult
            )
            gv01 = mid.tile([P, 2, ST], bf16, tag="gv01", bufs=5, name="gv01")
            thb = th[:].unsqueeze(1).broadcast_to([P, 2, ST])
            nc.vector.scalar_tensor_tensor(
                gv01[:], thb, 1.0, hv01[:], op0=ALU.add, op1=ALU.mult
            )
            # --- layer 2, skewed behind by `skew` supertiles so the PE
            # stream never waits on the activation/gate chain ---
            pending.append((act, gv01, gv2, k))
            if len(pending) > skew:
                emit_l2(pending.pop(0))

    for p in pending:
        emit_l2(p)


def build_nc(reps=1):
    bf16 = mybir.dt.bfloat16
    nc = bacc.Bacc("TRN2", target_bir_lowering=False, debug=False, num_devices=N_CORES)

    xt = nc.dram_tensor("xt", [4 * P, NP], bf16, kind="ExternalInput")
    wcat = nc.dram_tensor("wcat", [P, 640], bf16, kind="ExternalInput")
    outt = nc.dram_tensor("outt", [P, N_PAIR * ST], bf16, kind="ExternalOutput")

    with tile.TileContext(nc) as tc:
        with (
            tc.tile_pool(name="wsb", bufs=1) as wsb,
            # ring 5: 13 megatiles mod 3 == 1st slot, which would make each
            # hardware-loop rep's first DMA wait on the previous rep's tail
            # compute (full pipeline drain per rep); 13 mod 5 leaves slack
            tc.tile_pool(name="inp", bufs=5) as inp,
            tc.tile_pool(name="mid", bufs=2) as mid,
            tc.tile_pool(name="osb", bufs=3) as osbp,
            tc.tile_pool(name="ps", bufs=1, space="PSUM") as ps,
        ):
            w = wsb.tile([P, 640], bf16)
            nc.sync.dma_start(out=w[:], in_=wcat.ap())
            pools = (inp, mid, osbp, ps)
            if reps == 1:
                emit_body(nc, pools, xt.ap(), outt.ap(), w)
            else:
                # reps>1 exists only for the timing harness (bench.py):
                # repeat the whole kernel in a hardware loop so device time
                # dominates the ~100ms axon RPC dispatch overhead.
                with tc.For_i(0, reps, 1):
                    emit_body(nc, pools, xt.ap(), outt.ap(), w)

    nc.compile()
    return nc


# Row permutation: xt row f <- x column perm[f] (de-interleave vector irreps).
def _make_perm():
    perm = np.empty(512, np.int64)
    perm[:128] = np.arange(128)
    m = np.arange(128)
    for c in range(3):
        perm[128 + 128 * c + m] = 128 + 3 * m + c
    return perm


def _prep_weights(w1_s, w1_v, w2_s, w2_v):
    inv = np.float32(1.0 / np.sqrt(128.0))
    w2blk = np.zeros((128, 4, 64), np.float32)
    w2blk[:, 0, 0:16] = w2_s * inv
    for c in range(3):
        # 0.5 from the tanh-form sigmoid: gates = 0.5*(tanh(h/2)+1)
        w2blk[:, c + 1, 16 + 16 * c : 32 + 16 * c] = w2_v * (inv * np.float32(0.5))
    return np.ascontiguousarray(
        np.concatenate([w1_s * inv, w1_v * inv, w2blk.reshape(128, 256)], axis=1)
    ).astype(BF16)


def _prep_x_core(x, core, perm):
    lo = core * NC_NODES
    xt = np.zeros((512, NP), BF16)
    xt[:, :NC_NODES] = x[lo : lo + NC_NODES, perm].T.astype(BF16)
    return xt


def kernel(x, w1_s, w1_v, w2_s, w2_v):
    x = np.asarray(x, dtype=np.float32)
    wcat = _prep_weights(
        np.asarray(w1_s, np.float32),
        np.asarray(w1_v, np.float32),
        np.asarray(w2_s, np.float32),
        np.asarray(w2_v, np.float32),
    )
    perm = _make_perm()

    if "nc" not in _CACHE:
        _CACHE["nc"] = build_nc()
    nc = _CACHE["nc"]

    in_maps = [
        {"xt": _prep_x_core(x, core, perm), "wcat": wcat} for core in range(N_CORES)
    ]
    res = run_bass_kernel_spmd(nc, in_maps, core_ids=list(range(N_CORES)))

    out = np.empty((N_TOTAL, 64), np.float32)
    for core in range(N_CORES):
        lo = core * NC_NODES
        # outt[half*64 + ch, pair*512 + s] = out channel ch of node
        # 1024*pair + 512*half + s
        ot = np.asarray(res.results[core]["outt"], BF16).astype(np.float32)
        full = (
            ot.reshape(2, 64, N_PAIR, ST)
            .transpose(2, 0, 3, 1)
            .reshape(N_PAIR * 1024, 64)[:NC_NODES]
        )
        out[lo : lo + NC_NODES, :16] = full[:, :16]
        out[lo : lo + NC_NODES, 16:] = (
            full[:, 16:].reshape(NC_NODES, 3, 16).transpose(0, 2, 1).reshape(NC_NODES, 48)
        )
    return out


# revision 34
# speedup vs baseline: 1.0477x; 1.0477x over previous
"""Trainium2 Bass kernel for nn_NonLinearReadoutLayer (equivariant gated MLP readout).

Reference computation (per node, N=200000):
    s = x[:, :128]; v = x[:, 128:].reshape(N, 128, 3)
    h_s = (s @ w1_s) / sqrt(128)                # [N, 256]
    h_v = einsum('nmc,mk->nkc', v, w1_v) / sqrt(128)
    act = silu(h_s[:, :128]); gates = sigmoid(h_s[:, 128:])
    out_s = (act @ w2_s) / sqrt(128)            # [N, 16]
    out_v = einsum('nmc,mk->nkc', h_v * gates[:,:,None], w2_v) / sqrt(128)
    out = concat([out_s, out_v.reshape(N, 48)], 1)   # [N, 64]

Strategy: pure data-parallel over nodes across 8 cores. Host-side marshalling
puts x in feature-major layout xt[f, n] with the vector part de-interleaved
(f = 128 + 128*c + m) and CAST TO BF16 (halves the input HBM traffic, which
was the binding roofline at fp32; end-to-end absmax rel err ~4e-3 vs the
2e-2 gate). 1/sqrt(128) is folded into the weights. All on-chip ops are
[128]-contraction matmuls with nodes on the moving/free axis.

Pipeline: per 512-node supertile k the chain is L1 matmuls -> tanh/silu ->
gate-mul -> L2 matmuls. The in-order PE stream is software-pipelined with a
one-supertile skew (L1(k) ... L2(k-1)) so PE never waits on the activation
chain. Gate sigmoid is tanh-form (same ScalarE LUT set as Silu -> no table
reloads; the 0.5 folds into the layer-2 vector weights). (tanh+1)*h_v is
fused scalar_tensor_tensor on DVE, as one [128,2,512] op over a contiguous
two-bank h_v pair (stride-0 broadcast of tanh) plus one single — fewer
fixed-cost PSUM accesses. The four L2 matmuls of two consecutive supertiles
accumulate into ONE [128,512] PSUM bank (partitions 0:64 / 64:128), halving
evacuation cost; each pair is evacuated bf16 (ScalarE) and DMA'd out.
PSUM budget: hv01 ring2 (4) + hv2 (1) + hsa (1) + hsg (1) + po (1) = 8 banks.
"""

import os as _os_early
import numpy as np
import ml_dtypes

import concourse.mybir as mybir
import concourse.tile as tile
from concourse import bacc
from concourse.bass_utils import run_bass_kernel_spmd

N_CORES = 8
P = 128
ST = 512  # nodes per matmul group (one PSUM bank of fp32)
MT = int(_os_early.environ.get("K_MT", "2048"))  # nodes per DMA megatile
N_TOTAL = 200000
NC_NODES = N_TOTAL // N_CORES  # 25000
NP = 25088  # padded per-core nodes = 49 supertiles
N_ST = NP // ST  # 49
N_PAIR = (N_ST + 1) // 2  # 25 output pairs (last is half)

AF = mybir.ActivationFunctionType
ALU = mybir.AluOpType
BF16 = ml_dtypes.bfloat16

import os as _os

SPLIT_FIRST_MT = _os.environ.get("K_SPLIT_FIRST_MT", "1") == "1"
LDW_DEDUPE = _os.environ.get("K_LDW_DEDUPE", "0") == "1"


def _mm_noload(nc, out, lhsT, rhs, start, stop):
    """Matmul marked ldweights=False: PE array already holds lhsT from a
    preceding explicit nc.tensor.ldweights(lhsT). lhsT stays in ins so the
    dependency tracker still orders this after the weights are resident."""
    eng = nc.tensor
    ifmap_ap = eng.lower_ap(rhs.opt({0}), opt=False)
    weights_ap = eng.lower_ap(lhsT.opt({0}), opt=False, for_matmul_weights=True)
    out_ap = eng.lower_ap(out)
    return eng.add_instruction(
        mybir.InstMatmult(
            name=nc.get_next_instruction_name(),
            replication_resolution=0,
            replication_shift_amnt=0,
            replication_num_rows=0,
            start_tensor_calc=start,
            stop_tensor_calc=stop,
            ins=[ifmap_ap, weights_ap],
            outs=[out_ap],
            perf_mode=None,
            is_transpose=False,
            tile_position=(lhsT.base_partition(), out.base_partition()),
            tile_size=(128, 128),
            ldweights=False,
        )
    )

_CACHE = {}


def emit_body(nc, pools, xt_ap, out_ap, w):
    """One full pass over the node range. w is the preloaded weight tile."""
    f32 = mybir.dt.float32
    bf16 = mybir.dt.bfloat16
    inp, mid, osbp, ps = pools

    w1sa = w[:, 0:128]
    w1sb = w[:, 128:256]
    w1v = w[:, 256:384]
    w2 = [w[:, 384 + 64 * i : 448 + 64 * i] for i in range(4)]

    xt_r = xt_ap.rearrange("(b p) n -> p b n", p=P)

    state = {"po": None}
    skew = int(_os.environ.get("K_SKEW", "1"))

    def emit_l2(prev):
        act, gv01, gv2, k = prev
        if k % 2 == 0:
            state["po"] = ps.tile([P, ST], f32, tag="po", bufs=1, name="po")
        po = state["po"]
        prow = slice(64 * (k % 2), 64 * (k % 2) + 64)
        # accumulation order: act and gv2 first (their producers finish
        # earliest), the pair-fused gv01 last
        nc.tensor.matmul(po[prow, :], w2[0], act[:], start=True, stop=False)
        nc.tensor.matmul(po[prow, :], w2[3], gv2[:], start=False, stop=False)
        nc.tensor.matmul(po[prow, :], w2[1], gv01[:, 0, :], start=False, stop=False)
        nc.tensor.matmul(po[prow, :], w2[2], gv01[:, 1, :], start=False, stop=True)
        # per-supertile evacuation: each [64,512] half-bank frees on its own
        # schedule, so the po-bank WAR loop (last L2 mm -> evac -> next L2 mm)
        # spans two iterations instead of zero ring slack. The DMA shifts the
        # odd half back to partitions 0:64 (engines can't cross partitions).
        ost = osbp.tile([P, ST], bf16, tag="ost", bufs=5, name="ost")
        nc.scalar.copy(ost[prow, :], po[prow, :])
        nc.scalar.dma_start(
            out=out_ap[0:64, k * ST : (k + 1) * ST],
            in_=ost[prow, :],
        )

    pending = []
    for m0 in range(0, NP, MT):
        mt = min(MT, NP - m0)
        xin = inp.tile([P, 4, mt], bf16, tag="xin")
        if m0 == 0 and SPLIT_FIRST_MT:
            # split only the first megatile's DMA so compute starts after
            # ~1.6us (first supertile chunk) instead of the full 6us transfer
            for c0 in range(0, mt, ST):
                nc.sync.dma_start(
                    out=xin[:, :, c0 : c0 + ST], in_=xt_r[:, :, c0 : c0 + ST]
                )
        else:
            nc.sync.dma_start(out=xin[:], in_=xt_r[:, :, m0 : m0 + mt])

        for s0 in range(0, mt, ST):
            k = (m0 + s0) // ST
            sl = slice(s0, s0 + ST)
            # --- layer 1 matmuls: gate half first so tanh starts early ---
            h_sg = ps.tile([P, ST], f32, tag="hsg", bufs=1, name="hsg")
            nc.tensor.matmul(h_sg[:], w1sb, xin[:, 0, sl], start=True, stop=True)
            h_sa = ps.tile([P, ST], f32, tag="hsa", bufs=1, name="hsa")
            nc.tensor.matmul(h_sa[:], w1sa, xin[:, 0, sl], start=True, stop=True)
            hv01 = ps.tile([P, 2, ST], f32, tag="hv01", bufs=2, name="hv01")
            nc.tensor.matmul(hv01[:, 0, :], w1v, xin[:, 1, sl], start=True, stop=True)
            nc.tensor.matmul(hv01[:, 1, :], w1v, xin[:, 2, sl], start=True, stop=True)
            hv2 = ps.tile([P, ST], f32, tag="hv2", bufs=1, name="hv2")
            nc.tensor.matmul(hv2[:], w1v, xin[:, 3, sl], start=True, stop=True)
            # --- activations: sigmoid(x) = 0.5*(tanh(x/2)+1); 0.5 in w2v ---
            th = mid.tile([P, ST], bf16, tag="th", bufs=5, name="th")
            nc.scalar.activation(th[:], h_sg[:], AF.Tanh, scale=0.5)
            act = mid.tile([P, ST], bf16, tag="act", bufs=5, name="act")
            nc.scalar.activation(act[:], h_sa[:], AF.Silu)
            # --- gv = (th + 1) * h_v, fused on DVE; single (hv2) first so
            # its ring-1 bank frees before the next iteration's matmul ---
            gv2 = mid.tile([P, ST], bf16, tag="gv2", bufs=5, name="gv2")
            nc.vector.scalar_tensor_tensor(
                gv2[:], th[:], 1.0, hv2[:], op0=ALU.add, op1=ALU.mult
            )
            gv01 = mid.tile([P, 2, ST], bf16, tag="gv01", bufs=5, name="gv01")
            thb = th[:].unsqueeze(1).broadcast_to([P, 2, ST])
            nc.vector.scalar_tensor_tensor(
                gv01[:], thb, 1.0, hv01[:], op0=ALU.add, op1=ALU.mult
            )
            # --- layer 2, skewed behind by `skew` supertiles so the PE
            # stream never waits on the activation/gate chain ---
            pending.append((act, gv01, gv2, k))
            if len(pending) > skew:
                emit_l2(pending.pop(0))

    for p in pending:
        emit_l2(p)


def build_nc(reps=1):
    bf16 = mybir.dt.bfloat16
    nc = bacc.Bacc("TRN2", target_bir_lowering=False, debug=False, num_devices=N_CORES)

    xt = nc.dram_tensor("xt", [4 * P, NP], bf16, kind="ExternalInput")
    wcat = nc.dram_tensor("wcat", [P, 640], bf16, kind="ExternalInput")
    outt = nc.dram_tensor("outt", [64, NP], bf16, kind="ExternalOutput")

    with tile.TileContext(nc) as tc:
        with (
            tc.tile_pool(name="wsb", bufs=1) as wsb,
            # ring 5: 13 megatiles mod 3 == 1st slot, which would make each
            # hardware-loop rep's first DMA wait on the previous rep's tail
            # compute (full pipeline drain per rep); 13 mod 5 leaves slack
            tc.tile_pool(name="inp", bufs=4 if MT == 4096 else 5) as inp,
            tc.tile_pool(name="mid", bufs=2) as mid,
            tc.tile_pool(name="osb", bufs=3) as osbp,
            tc.tile_pool(name="ps", bufs=1, space="PSUM") as ps,
        ):
            w = wsb.tile([P, 640], bf16)
            nc.sync.dma_start(out=w[:], in_=wcat.ap())
            pools = (inp, mid, osbp, ps)
            if reps == 1:
                emit_body(nc, pools, xt.ap(), outt.ap(), w)
            else:
                # reps>1 exists only for the timing harness (bench.py):
                # repeat the whole kernel in a hardware loop so device time
                # dominates the ~100ms axon RPC dispatch overhead.
                with tc.For_i(0, reps, 1):
                    emit_body(nc, pools, xt.ap(), outt.ap(), w)

    nc.compile()
    return nc


# Row permutation: xt row f <- x column perm[f] (de-interleave vector irreps).
def _make_perm():
    perm = np.empty(512, np.int64)
    perm[:128] = np.arange(128)
    m = np.arange(128)
    for c in range(3):
        perm[128 + 128 * c + m] = 128 + 3 * m + c
    return perm


def _prep_weights(w1_s, w1_v, w2_s, w2_v):
    inv = np.float32(1.0 / np.sqrt(128.0))
    w2blk = np.zeros((128, 4, 64), np.float32)
    w2blk[:, 0, 0:16] = w2_s * inv
    for c in range(3):
        # 0.5 from the tanh-form sigmoid: gates = 0.5*(tanh(h/2)+1)
        w2blk[:, c + 1, 16 + 16 * c : 32 + 16 * c] = w2_v * (inv * np.float32(0.5))
    return np.ascontiguousarray(
        np.concatenate([w1_s * inv, w1_v * inv, w2blk.reshape(128, 256)], axis=1)
    ).astype(BF16)


def _prep_x_core(x, core, perm):
    lo = core * NC_NODES
    xt = np.zeros((512, NP), BF16)
    xt[:, :NC_NODES] = x[lo : lo + NC_NODES, perm].T.astype(BF16)
    return xt


def kernel(x, w1_s, w1_v, w2_s, w2_v):
    x = np.asarray(x, dtype=np.float32)
    wcat = _prep_weights(
        np.asarray(w1_s, np.float32),
        np.asarray(w1_v, np.float32),
        np.asarray(w2_s, np.float32),
        np.asarray(w2_v, np.float32),
    )
    perm = _make_perm()

    if "nc" not in _CACHE:
        _CACHE["nc"] = build_nc()
    nc = _CACHE["nc"]

    in_maps = [
        {"xt": _prep_x_core(x, core, perm), "wcat": wcat} for core in range(N_CORES)
    ]
    res = run_bass_kernel_spmd(nc, in_maps, core_ids=list(range(N_CORES)))

    out = np.empty((N_TOTAL, 64), np.float32)
    for core in range(N_CORES):
        lo = core * NC_NODES
        # outt[ch, n] = out channel ch of node n
        ot = np.asarray(res.results[core]["outt"], BF16).astype(np.float32)
        full = ot[:, :NC_NODES].T
        out[lo : lo + NC_NODES, :16] = full[:, :16]
        out[lo : lo + NC_NODES, 16:] = (
            full[:, 16:].reshape(NC_NODES, 3, 16).transpose(0, 2, 1).reshape(NC_NODES, 48)
        )
    return out
